# revision 1
# baseline (speedup 1.0000x reference)
"""Trainium2 Bass kernel for nn_CrossModalFusion (single-head cross attention).

Per-batch-element cross attention, data-parallel over B=8 across 8 NeuronCores.

Per core (T=2048, D_RGB=400, D_POSE=256, H=512):
    q = rgb @ Wq + bq ; k = pose @ Wk + bk ; v = pose @ Wv
    S = q @ k.T / sqrt(H) ; A = exp(S) (no max-sub needed; scores are O(1))
    y = rgb + bp + bv@Wp + (A @ v) @ Wp / rowsum(A)

Layout strategy (zero on-device transposes):
  - host feeds rgb^T (d padded 400->512), pose^T in fp8e4m3 so projections
    contract d on partitions with DoubleRow (2 MACs/cell/cycle)
  - qT,kT computed h-major [h,t]; scores computed transposed ST=[tk,tq]
  - exp(ST - ln 32) on ACT (the 1/32 keeps unnormalized O inside fp8e4m3
    range; it cancels in the normalization); O^T accumulated via lhsT=v
    (natural layout), rhs=exp(ST); row-sums via a ones-vector matmul
  - every matmul uses fp8e4m3 operands with perf_mode=DoubleRow (pairs of
    128-row k-subtiles), fp32 PSUM accumulation everywhere
  - device returns unnormalized (A@v)@Wp and rowsum(A); the host applies
    y = rgb + bp' + yun/sums in fp32 (0.1% of the FLOPs, exact division)
"""

import sys

if "/opt/trn_rl_repo" not in sys.path:
    sys.path.insert(0, "/opt/trn_rl_repo")

from contextlib import ExitStack

import ml_dtypes
import numpy as np

import concourse.mybir as mybir
import concourse.tile as tile
from concourse import bacc, bass_utils

FP8 = mybir.dt.float8e4
F32 = mybir.dt.float32
NP_FP8 = ml_dtypes.float8_e4m3

B, T, DR, DP, H = 8, 2048, 400, 256, 512
PART = 128
DRP = 512                # rgb feature dim padded to 4*128
TQC = 512                # tq chunk width (max PSUM free dim)
NCH = T // TQC           # 4 chunks
NTK = T // PART          # 16 key tiles
NKP = NTK // 2           # 8 key tile pairs (DoubleRow)
NHT = H // PART          # 4 h tiles
NHP = NHT // 2           # 2 h tile pairs
NDR = DRP // PART        # 4 padded-rgb d tiles
NDRP = NDR // 2          # 2 pairs
NDP = DP // PART         # 2 pose d tiles
SCALE = float(1.0 / np.sqrt(np.float32(H)))
EXP_BIAS = float(-np.log(32.0))

AT = mybir.ActivationFunctionType
OP = mybir.AluOpType
DRM = mybir.MatmulPerfMode.DoubleRow


def build_nc():
    nc = bacc.Bacc(
        "TRN2",
        target_bir_lowering=False,
        debug=False,
        enable_asserts=False,
        num_devices=8,
    )
    xT = nc.dram_tensor("xT", (DRP, T), FP8, kind="ExternalInput").ap()
    pT = nc.dram_tensor("pT", (DP, T), FP8, kind="ExternalInput").ap()
    wq = nc.dram_tensor("wq", (DRP, H), FP8, kind="ExternalInput").ap()
    wk = nc.dram_tensor("wk", (DP, H), FP8, kind="ExternalInput").ap()
    wv = nc.dram_tensor("wv", (DP, H), FP8, kind="ExternalInput").ap()
    wp = nc.dram_tensor("wp", (H, DR), FP8, kind="ExternalInput").ap()
    bqc = nc.dram_tensor("bqc", (PART, NHT), F32, kind="ExternalInput").ap()
    bkc = nc.dram_tensor("bkc", (PART, NHT), F32, kind="ExternalInput").ap()
    yun = nc.dram_tensor("yun", (T, DR), F32, kind="ExternalOutput").ap()
    sums_out = nc.dram_tensor("sums_out", (NCH, TQC), F32, kind="ExternalOutput").ap()

    with tile.TileContext(nc) as tc, ExitStack() as ctx:
        const = ctx.enter_context(tc.tile_pool(name="const", bufs=1))
        mm_ps = ctx.enter_context(tc.tile_pool(name="mm_ps", bufs=3, space="PSUM"))
        ot_ps = ctx.enter_context(tc.tile_pool(name="ot_ps", bufs=4, space="PSUM"))
        sum_ps = ctx.enter_context(tc.tile_pool(name="sum_ps", bufs=1, space="PSUM"))
        ex_pool = ctx.enter_context(tc.tile_pool(name="ex_pool", bufs=10))
        sums_pool = ctx.enter_context(tc.tile_pool(name="sums_pool", bufs=2))
        ysb_pool = ctx.enter_context(tc.tile_pool(name="ysb_pool", bufs=4))

        # ---- persistent inputs ----
        # big streams on the sync HWDGE queue; small weights/biases in
        # parallel on the scalar HWDGE queue, kT path first on both
        wk8 = const.tile([PART, NDP, H], FP8, name="wk8")
        nc.scalar.dma_start(wk8[:], wk.rearrange("(k p) h -> p k h", p=PART))
        wv8 = const.tile([PART, NDP, H], FP8, name="wv8")
        nc.scalar.dma_start(wv8[:], wv.rearrange("(k p) h -> p k h", p=PART))
        wq8 = const.tile([PART, NDR, H], FP8, name="wq8")
        nc.scalar.dma_start(wq8[:], wq.rearrange("(k p) h -> p k h", p=PART))
        wp8 = const.tile([PART, NHT, DR], FP8, name="wp8")
        nc.scalar.dma_start(wp8[:], wp.rearrange("(k p) d -> p k d", p=PART))
        # descriptor-heavy tiny loads go on the otherwise-idle gpsimd queue
        bq_sb = const.tile([PART, NHT], F32, name="bq_sb")
        nc.gpsimd.dma_start(bq_sb[:], bqc[:])
        bk_sb = const.tile([PART, NHT], F32, name="bk_sb")
        nc.gpsimd.dma_start(bk_sb[:], bkc[:])
        p8 = const.tile([PART, NDP, T], FP8, name="p8")
        for h in range(4):  # split for earlier first-chunk availability
            nc.sync.dma_start(
                p8[:, :, h * (T // 4) : (h + 1) * (T // 4)],
                pT[:, h * (T // 4) : (h + 1) * (T // 4)].rearrange(
                    "(k p) t -> p k t", p=PART
                ),
            )
        x8 = const.tile([PART, NDR, T], FP8, name="x8")
        for h, eng in ((0, nc.sync), (1, nc.scalar)):
            eng.dma_start(
                x8[:, :, h * (T // 2) : (h + 1) * (T // 2)],
                xT[:, h * (T // 2) : (h + 1) * (T // 2)].rearrange(
                    "(k p) t -> p k t", p=PART
                ),
            )
        ones8 = const.tile([PART, 2, 16], FP8, name="ones8")
        nc.vector.memset(ones8[:], 1.0)
        expb = const.tile([PART, 1], F32, name="expb")
        nc.vector.memset(expb[:], EXP_BIAS)

        # ---- persistent intermediates (fp8 DoubleRow pair layouts) ----
        # qT8[i2][p, s, t] = q[h = i2*256 + s*128 + p, t]
        qT8 = [const.tile([PART, 2, T], FP8, name=f"qT8_{i}") for i in range(NHP)]
        kT8 = [const.tile([PART, 2, T], FP8, name=f"kT8_{i}") for i in range(NHP)]
        # v8[j2][p, s, h] = v[t = j2*256 + s*128 + p, h]
        v8 = [const.tile([PART, 2, H], FP8, name=f"v8_{j}") for j in range(NKP)]
        # ot8[i2][p, s, t] = O[h = i2*256 + s*128 + p, t] (unnormalized, /32)
        ot8 = [const.tile([PART, 2, T], FP8, name=f"ot8_{i}") for i in range(NHP)]

        def evict_biased(n, dst, ps, bias_ap, scale):
            """PSUM->SBUF cast with scale*x+bias, alternating DVE/ACT."""
            if n % 2 == 0:
                if scale == 1.0:
                    nc.vector.tensor_scalar_add(dst, ps, bias_ap)
                else:
                    nc.vector.tensor_scalar(
                        dst, ps, scale, bias_ap, op0=OP.mult, op1=OP.add
                    )
            else:
                nc.scalar.activation(dst, ps, AT.Identity, bias=bias_ap, scale=scale)

        # ---- phase B: projections (all DoubleRow over d pairs) ----
        # kT[h,t] = (Wk[d,h].T @ pT[d,t]) * scale + bk*scale -> fp8
        # (c-outer: chunks 0/1 only need the first half of p8)
        for c in range(NCH):
            for i in range(NHT):
                ps = mm_ps.tile([PART, TQC], F32, name=f"kps_{i}_{c}", tag="mmps")
                nc.tensor.matmul(
                    ps[:],
                    wk8[:, :, i * PART : (i + 1) * PART],
                    p8[:, :, c * TQC : (c + 1) * TQC],
                    start=True,
                    stop=True,
                    perf_mode=DRM,
                )
                evict_biased(
                    c * NHT + i,
                    kT8[i // 2][:, i % 2, c * TQC : (c + 1) * TQC],
                    ps[:],
                    bk_sb[:, i : i + 1],
                    SCALE,
                )
        # v[t,h] = pT[d,t].T @ Wv[d,h] -> fp8
        for j in range(NTK):
            ps = mm_ps.tile([PART, H], F32, name=f"vps_{j}", tag="mmps")
            nc.tensor.matmul(
                ps[:],
                p8[:, :, j * PART : (j + 1) * PART],
                wv8[:],
                start=True,
                stop=True,
                perf_mode=DRM,
            )
            if j % 2 == 0:
                nc.scalar.copy(v8[j // 2][:, j % 2, :], ps[:])
            else:
                nc.vector.tensor_copy(v8[j // 2][:, j % 2, :], ps[:])
        # qT[h,t] = Wq[d,h].T @ xT[d,t] + bq -> fp8 (c-outer so chunk 0 is ready early)
        for c in range(NCH):
            for i in range(NHT):
                ps = mm_ps.tile([PART, TQC], F32, name=f"qps_{i}_{c}", tag="mmps")
                for d2 in range(NDRP):
                    nc.tensor.matmul(
                        ps[:],
                        wq8[:, 2 * d2 : 2 * d2 + 2, i * PART : (i + 1) * PART],
                        x8[:, 2 * d2 : 2 * d2 + 2, c * TQC : (c + 1) * TQC],
                        start=(d2 == 0),
                        stop=(d2 == NDRP - 1),
                        perf_mode=DRM,
                    )
                evict_biased(
                    c * NHT + i + 1,
                    qT8[i // 2][:, i % 2, c * TQC : (c + 1) * TQC],
                    ps[:],
                    bq_sb[:, i : i + 1],
                    1.0,
                )

        # ---- phase C: attention, chunked over tq ----
        # phase D (output projection) for chunk c-1 is emitted a few j-steps
        # into chunk c so its PSUM/engine traffic doesn't cluster at the
        # chunk boundary.
        def emit_y_tile(c, tl):
            tg = c * (TQC // PART) + tl
            yp = mm_ps.tile([PART, DR], F32, name=f"yp_{tg}", tag="mmps")
            for i2 in range(NHP):
                nc.tensor.matmul(
                    yp[:],
                    ot8[i2][:, :, tg * PART : (tg + 1) * PART],
                    wp8[:, 2 * i2 : 2 * i2 + 2, :],
                    start=(i2 == 0),
                    stop=(i2 == NHP - 1),
                    perf_mode=DRM,
                )
            ysb = ysb_pool.tile([PART, DR], F32, name=f"ysb_{tg}", tag="ysb")
            # alternate engines so the final chain parallelizes at kernel end
            if tl % 2 == 0:
                nc.vector.tensor_copy(ysb[:], yp[:])
                nc.sync.dma_start(yun[tg * PART : (tg + 1) * PART, :], ysb[:])
            else:
                nc.scalar.copy(ysb[:], yp[:])
                nc.scalar.dma_start(yun[tg * PART : (tg + 1) * PART, :], ysb[:])

        for c in range(NCH):
            otps = [
                ot_ps.tile([PART, TQC], F32, name=f"otp_{c}_{i}", tag="otp")
                for i in range(NHT)
            ]
            sps = sum_ps.tile([1, TQC], F32, name=f"sump_{c}", tag="sump")
            exs = []
            for j in range(NTK):
                st = mm_ps.tile([PART, TQC], F32, name=f"st_{c}_{j}", tag="mmps")
                for i2 in range(NHP):
                    nc.tensor.matmul(
                        st[:],
                        kT8[i2][:, :, j * PART : (j + 1) * PART],
                        qT8[i2][:, :, c * TQC : (c + 1) * TQC],
                        start=(i2 == 0),
                        stop=(i2 == NHP - 1),
                        perf_mode=DRM,
                    )
                if j % 2 == 0:
                    ex = ex_pool.tile([PART, 2, TQC], FP8, name=f"ex_{c}_{j}", tag="ex")
                    exs.append(ex)
                nc.scalar.activation(
                    exs[-1][:, j % 2, :], st[:], AT.Exp, bias=expb[:]
                )
                # y tiles of the previous chunk, spread through this chunk so
                # they don't crowd the mmps PSUM slots at the boundary; the
                # last one fills the PE bubble while the final exp is on ACT
                if c > 0 and j in (5, 9, 13, 15):
                    emit_y_tile(c - 1, (5, 9, 13, 15).index(j))
                if j % 2 == 1:
                    j2 = j // 2
                    ex = exs[-1]
                    # sums first: its 2-column LDWEIGHTS gives the weight-load
                    # port slack between the 256-column v-slice loads
                    nc.tensor.matmul(
                        sps[:],
                        ones8[:, :, 0:1],
                        ex[:],
                        start=(j2 == 0),
                        stop=(j2 == NKP - 1),
                        perf_mode=DRM,
                    )
                    for i in range(NHT):
                        nc.tensor.matmul(
                            otps[i][:],
                            v8[j2][:, :, i * PART : (i + 1) * PART],
                            ex[:],
                            start=(j2 == 0),
                            stop=(j2 == NKP - 1),
                            perf_mode=DRM,
                        )
            for i in range(NHT):
                # split across ACT/DVE so neither engine bursts at the boundary
                dst = ot8[i // 2][:, i % 2, c * TQC : (c + 1) * TQC]
                if i % 2 == 0:
                    nc.scalar.copy(dst, otps[i][:])
                else:
                    nc.vector.tensor_copy(dst, otps[i][:])
            sums_sb = sums_pool.tile([1, TQC], F32, name=f"sums_{c}", tag="sums")
            nc.vector.tensor_copy(sums_sb[:], sps[:])
            nc.sync.dma_start(sums_out[c : c + 1, :], sums_sb[:])

        for tl in range(TQC // PART):
            emit_y_tile(NCH - 1, tl)

    nc.compile()
    return nc


_NC_CACHE = None


def get_nc():
    global _NC_CACHE
    if _NC_CACHE is None:
        _NC_CACHE = build_nc()
    return _NC_CACHE


def make_in_maps(rgb, pose, Wq, bq, Wk, bk, Wv, bv, Wp, bp):
    rgb = np.asarray(rgb, np.float32)
    pose = np.asarray(pose, np.float32)
    Wq, bq = np.asarray(Wq, np.float32), np.asarray(bq, np.float32)
    Wk, bk = np.asarray(Wk, np.float32), np.asarray(bk, np.float32)
    Wv = np.asarray(Wv, np.float32)
    Wp = np.asarray(Wp, np.float32)

    xT = np.zeros((B, DRP, T), NP_FP8)
    xT[:, :DR, :] = np.swapaxes(rgb, 1, 2).astype(NP_FP8)
    pT = np.ascontiguousarray(np.swapaxes(pose, 1, 2)).astype(NP_FP8)
    wq8 = np.zeros((DRP, H), NP_FP8)
    wq8[:DR] = Wq.astype(NP_FP8)
    wk8 = Wk.astype(NP_FP8)
    wv8 = Wv.astype(NP_FP8)
    wp8 = Wp.astype(NP_FP8)
    bqc = np.ascontiguousarray(bq.reshape(NHT, PART).T).astype(np.float32)
    bkc = np.ascontiguousarray((bk * SCALE).reshape(NHT, PART).T).astype(np.float32)
    return [
        dict(
            xT=xT[b], pT=pT[b],
            wq=wq8, wk=wk8, wv=wv8, wp=wp8, bqc=bqc, bkc=bkc,
        )
        for b in range(B)
    ]


def kernel(rgb, pose, Wq, bq, Wk, bk, Wv, bv, Wp, bp):
    rgb = np.asarray(rgb, np.float32)
    Wp_f = np.asarray(Wp, np.float32)
    bp_eff = np.asarray(bp, np.float32) + np.asarray(bv, np.float32) @ Wp_f
    in_maps = make_in_maps(rgb, pose, Wq, bq, Wk, bk, Wv, bv, Wp, bp)
    res = bass_utils.run_bass_kernel_spmd(get_nc(), in_maps, core_ids=list(range(B)))
    out = np.empty((B, T, DR), np.float32)
    for b in range(B):
        yun = res.results[b]["yun"]
        sums = res.results[b]["sums_out"].reshape(T)
        out[b] = rgb[b] + bp_eff + yun / sums[:, None]
    return out



# revision 5
# speedup vs baseline: 1.0200x; 1.0200x over previous
"""Trainium2 Bass kernel for nn_CrossModalFusion (single-head cross attention).

Per-batch-element cross attention, data-parallel over B=8 across 8 NeuronCores.

Per core (T=2048, D_RGB=400, D_POSE=256, H=512):
    q = rgb @ Wq + bq ; k = pose @ Wk + bk ; v = pose @ Wv
    S = q @ k.T / sqrt(H) ; A = exp(S) (no max-sub needed; scores are O(1))
    y = rgb + bp + bv@Wp + (A @ v) @ Wp / rowsum(A)

Layout strategy (zero on-device transposes):
  - host pre-permutes every tensor into its exact SBUF layout so each DMA is
    a flat contiguous copy; inputs ordered across the sync/scalar/gpsimd
    queues to match first-use times (per-queue DMA sustains only ~50GB/s)
  - warm-up matmuls on a memset tile burn the PE p-state ramp (0.65->2.4GHz)
    inside the input-DMA window
  - qT,kT computed h-major [h,t]; scores computed transposed ST=[tk,tq]
  - exp(ST - ln 32) on ACT (the 1/32 keeps unnormalized O inside fp8e4m3
    range; it cancels in the normalization); O^T accumulated via lhsT=v
    (natural layout), rhs=exp(ST); row-sums via a ones-vector matmul
  - every matmul uses fp8e4m3 operands with perf_mode=DoubleRow (pairs of
    128-row k-subtiles), fp32 PSUM accumulation everywhere
  - the O/sums accumulation for score-pair p is emitted after the scores of
    pair p+1 (one-pair software pipeline); the final pair's O group lands
    after the NEXT chunk's first two score tiles, so the PE never head-of-line
    blocks on the chunk's last exp
  - device returns unnormalized (A@v)@Wp in BF16 (halves the exposed tail
    DMA) and rowsum(A) in fp32; the host applies
    y = rgb + bp' + yun/sums in fp32 (0.1% of the FLOPs, exact division)
"""

import sys

if "/opt/trn_rl_repo" not in sys.path:
    sys.path.insert(0, "/opt/trn_rl_repo")

from contextlib import ExitStack

import ml_dtypes
import numpy as np

import concourse.mybir as mybir
import concourse.tile as tile
from concourse import bacc, bass_utils

FP8 = mybir.dt.float8e4
F32 = mybir.dt.float32
BF16 = mybir.dt.bfloat16
NP_FP8 = ml_dtypes.float8_e4m3

B, T, DR, DP, H = 8, 2048, 400, 256, 512
PART = 128
DRP = 512                # rgb feature dim padded to 4*128
TQC = 512                # tq chunk width (max PSUM free dim)
NCH = T // TQC           # 4 chunks
NTK = T // PART          # 16 key tiles
NKP = NTK // 2           # 8 key tile pairs (DoubleRow)
NHT = H // PART          # 4 h tiles
NHP = NHT // 2           # 2 h tile pairs
NDR = DRP // PART        # 4 padded-rgb d tiles
NDRP = NDR // 2          # 2 pairs
NDP = DP // PART         # 2 pose d tiles
SCALE = float(1.0 / np.sqrt(np.float32(H)))
EXP_BIAS = float(-np.log(32.0))

AT = mybir.ActivationFunctionType
OP = mybir.AluOpType
DRM = mybir.MatmulPerfMode.DoubleRow


def build_nc():
    nc = bacc.Bacc(
        "TRN2",
        target_bir_lowering=False,
        debug=False,
        enable_asserts=False,
        num_devices=8,
    )
    # all inputs arrive pre-permuted from the host: flat contiguous DMAs only
    xq_d = [
        nc.dram_tensor(f"xq{c}", (PART, NDR, TQC), FP8, kind="ExternalInput").ap()
        for c in range(NCH)
    ]
    pq_d = [
        nc.dram_tensor(f"pq{c}", (PART, NDP, TQC), FP8, kind="ExternalInput").ap()
        for c in range(NCH)
    ]
    wka_d = nc.dram_tensor("wka", (PART, NDP, 2 * PART), FP8, kind="ExternalInput").ap()
    wkb_d = nc.dram_tensor("wkb", (PART, NDP, 2 * PART), FP8, kind="ExternalInput").ap()
    wva_d = nc.dram_tensor("wva", (PART, 1, H), FP8, kind="ExternalInput").ap()
    wvb_d = nc.dram_tensor("wvb", (PART, 1, H), FP8, kind="ExternalInput").ap()
    wq_d = nc.dram_tensor("wq", (PART, NDR, H), FP8, kind="ExternalInput").ap()
    wp_d = nc.dram_tensor("wp", (PART, NHT, DR), FP8, kind="ExternalInput").ap()
    bqbk_d = nc.dram_tensor("bqbk", (PART, 2 * NHT), F32, kind="ExternalInput").ap()
    yun = nc.dram_tensor("yun", (T, DR), BF16, kind="ExternalOutput").ap()
    sums_out = nc.dram_tensor("sums_out", (NCH, TQC), F32, kind="ExternalOutput").ap()

    with tile.TileContext(nc) as tc, ExitStack() as ctx:
        const = ctx.enter_context(tc.tile_pool(name="const", bufs=1))
        mm_ps = ctx.enter_context(tc.tile_pool(name="mm_ps", bufs=3, space="PSUM"))
        ot_ps = ctx.enter_context(tc.tile_pool(name="ot_ps", bufs=4, space="PSUM"))
        sum_ps = ctx.enter_context(tc.tile_pool(name="sum_ps", bufs=1, space="PSUM"))
        ex_pool = ctx.enter_context(tc.tile_pool(name="ex_pool", bufs=10))
        sums_pool = ctx.enter_context(tc.tile_pool(name="sums_pool", bufs=2))
        ysb_pool = ctx.enter_context(tc.tile_pool(name="ysb_pool", bufs=4))

        # ---- persistent inputs ----
        # flat contiguous DMAs, ordered per queue by first-use time:
        #   scalar: wka, wv half a, wq, xq2, wp
        #   sync:   pq0..pq3, xq3
        #   gpsimd: bqbk, wkb, wv half b, xq0, xq1
        wk8 = [const.tile([PART, NDP, 2 * PART], FP8, name=f"wk8_{i}") for i in range(2)]
        wv8 = const.tile([PART, NDP, H], FP8, name="wv8")
        wq8 = const.tile([PART, NDR, H], FP8, name="wq8")
        wp8 = const.tile([PART, NHT, DR], FP8, name="wp8")
        p8 = [const.tile([PART, NDP, TQC], FP8, name=f"p8_{c}") for c in range(NCH)]
        x8 = [const.tile([PART, NDR, TQC], FP8, name=f"x8_{c}") for c in range(NCH)]
        bqbk = const.tile([PART, 2 * NHT], F32, name="bqbk")

        nc.scalar.dma_start(wk8[0][:], wka_d[:])
        nc.scalar.dma_start(wv8[:, 0:1, :], wva_d[:])
        nc.scalar.dma_start(wq8[:], wq_d[:])
        nc.scalar.dma_start(x8[2][:], xq_d[2][:])
        nc.scalar.dma_start(wp8[:], wp_d[:])

        for c in range(NCH):
            nc.sync.dma_start(p8[c][:], pq_d[c][:])
        nc.sync.dma_start(x8[3][:], xq_d[3][:])

        nc.gpsimd.dma_start(bqbk[:], bqbk_d[:])
        nc.gpsimd.dma_start(wk8[1][:], wkb_d[:])
        nc.gpsimd.dma_start(wv8[:, 1:2, :], wvb_d[:])
        nc.gpsimd.dma_start(x8[0][:], xq_d[0][:])
        nc.gpsimd.dma_start(x8[1][:], xq_d[1][:])

        ones8 = const.tile([PART, 2, PART], FP8, name="ones8")
        nc.vector.memset(ones8[:], 1.0)
        expb = const.tile([PART, 1], F32, name="expb")
        nc.vector.memset(expb[:], EXP_BIAS)

        # ---- persistent intermediates (fp8 DoubleRow pair layouts) ----
        # qT8[i2][p, s, t] = q[h = i2*256 + s*128 + p, t]
        qT8 = [const.tile([PART, 2, T], FP8, name=f"qT8_{i}") for i in range(NHP)]
        kT8 = [const.tile([PART, 2, T], FP8, name=f"kT8_{i}") for i in range(NHP)]
        # v8[j2][p, s, h] = v[t = j2*256 + s*128 + p, h]
        v8 = [const.tile([PART, 2, H], FP8, name=f"v8_{j}") for j in range(NKP)]
        # ot8[i2][p, s, t] = O[h = i2*256 + s*128 + p, t] (unnormalized, /32)
        ot8 = [const.tile([PART, 2, T], FP8, name=f"ot8_{i}") for i in range(NHP)]

        # ---- PE warm-up: burn the p-state ramp inside the DMA window ----
        for w in range(8):
            wps = mm_ps.tile([PART, PART], F32, name=f"warm_{w}", tag="mmps")
            nc.tensor.matmul(
                wps[:], ones8[:], ones8[:], start=True, stop=True, perf_mode=DRM
            )

        def evict_biased(n, dst, ps, bias_ap, scale):
            """PSUM->SBUF cast with scale*x+bias, alternating DVE/ACT."""
            if n % 2 == 0:
                if scale == 1.0:
                    nc.vector.tensor_scalar_add(dst, ps, bias_ap)
                else:
                    nc.vector.tensor_scalar(
                        dst, ps, scale, bias_ap, op0=OP.mult, op1=OP.add
                    )
            else:
                nc.scalar.activation(dst, ps, AT.Identity, bias=bias_ap, scale=scale)

        # ---- phase B: projections (all DoubleRow over d pairs) ----
        # kT[h,t] = (Wk[d,h].T @ pT[d,t]) * scale + bk*scale -> fp8
        # v[t,h] = pT[d,t].T @ Wv[d,h] -> fp8
        # kT chunk c and v tiles 4c..4c+3 both depend only on pose quarter c,
        # so interleave them to track the sync-queue arrivals.
        for c in range(NCH):
            for i in range(NHT):
                ps = mm_ps.tile([PART, TQC], F32, name=f"kps_{i}_{c}", tag="mmps")
                nc.tensor.matmul(
                    ps[:],
                    wk8[i // 2][:, :, (i % 2) * PART : (i % 2 + 1) * PART],
                    p8[c][:],
                    start=True,
                    stop=True,
                    perf_mode=DRM,
                )
                evict_biased(
                    c * NHT + i,
                    kT8[i // 2][:, i % 2, c * TQC : (c + 1) * TQC],
                    ps[:],
                    bqbk[:, NHT + i : NHT + i + 1],
                    SCALE,
                )
            for jl in range(NCH):
                j = c * NCH + jl
                ps = mm_ps.tile([PART, H], F32, name=f"vps_{j}", tag="mmps")
                nc.tensor.matmul(
                    ps[:],
                    p8[c][:, :, jl * PART : (jl + 1) * PART],
                    wv8[:],
                    start=True,
                    stop=True,
                    perf_mode=DRM,
                )
                if j % 2 == 0:
                    nc.scalar.copy(v8[j // 2][:, j % 2, :], ps[:])
                else:
                    nc.vector.tensor_copy(v8[j // 2][:, j % 2, :], ps[:])
        # qT[h,t] = Wq[d,h].T @ xT[d,t] + bq -> fp8 (c-outer so chunk 0 is ready early)
        for c in range(NCH):
            for i in range(NHT):
                ps = mm_ps.tile([PART, TQC], F32, name=f"qps_{i}_{c}", tag="mmps")
                for d2 in range(NDRP):
                    nc.tensor.matmul(
                        ps[:],
                        wq8[:, 2 * d2 : 2 * d2 + 2, i * PART : (i + 1) * PART],
                        x8[c][:, 2 * d2 : 2 * d2 + 2, :],
                        start=(d2 == 0),
                        stop=(d2 == NDRP - 1),
                        perf_mode=DRM,
                    )
                evict_biased(
                    c * NHT + i + 1,
                    qT8[i // 2][:, i % 2, c * TQC : (c + 1) * TQC],
                    ps[:],
                    bqbk[:, i : i + 1],
                    1.0,
                )

        # ---- phase C: attention, chunked over tq ----
        # Software pipeline: the O/sums group for score-pair p is emitted
        # after pair p+1's scores; the last pair's group lands after the next
        # chunk's first two score tiles.  Output projection (phase D) for
        # chunk c-1 is spread through chunk c at pairs 2..5.
        def emit_y_tile(c, tl, last=False):
            tg = c * (TQC // PART) + tl
            yp = mm_ps.tile([PART, DR], F32, name=f"yp_{tg}", tag="mmps")
            for i2 in range(NHP):
                nc.tensor.matmul(
                    yp[:],
                    ot8[i2][:, :, tg * PART : (tg + 1) * PART],
                    wp8[:, 2 * i2 : 2 * i2 + 2, :],
                    start=(i2 == 0),
                    stop=(i2 == NHP - 1),
                    perf_mode=DRM,
                )
            ysb = ysb_pool.tile([PART, DR], BF16, name=f"ysb_{tg}", tag="ysb")
            # alternate engines so the final chain parallelizes at kernel end
            if tl % 2 == 0:
                nc.vector.tensor_copy(ysb[:], yp[:])
            else:
                nc.scalar.copy(ysb[:], yp[:])
            dst = yun[tg * PART : (tg + 1) * PART, :]
            if last:
                # final chunk: halve each transfer across two queues so the
                # very last tile's data is exposed for only ~1us
                e0, e1 = ((nc.sync, nc.scalar), (nc.scalar, nc.gpsimd),
                          (nc.gpsimd, nc.sync), (nc.sync, nc.scalar))[tl]
                e0.dma_start(dst[0 : PART // 2, :], ysb[0 : PART // 2, :])
                e1.dma_start(dst[PART // 2 : PART, :], ysb[PART // 2 : PART, :])
            else:
                (nc.sync, nc.scalar)[tl % 2].dma_start(dst, ysb[:])

        chunk_state = {}  # c -> (otps, sps, exs)

        def emit_scores_pair(c, p):
            exs = chunk_state[c][2]
            for j in (2 * p, 2 * p + 1):
                st = mm_ps.tile([PART, TQC], F32, name=f"st_{c}_{j}", tag="mmps")
                for i2 in range(NHP):
                    nc.tensor.matmul(
                        st[:],
                        kT8[i2][:, :, j * PART : (j + 1) * PART],
                        qT8[i2][:, :, c * TQC : (c + 1) * TQC],
                        start=(i2 == 0),
                        stop=(i2 == NHP - 1),
                        perf_mode=DRM,
                    )
                if j % 2 == 0:
                    ex = ex_pool.tile([PART, 2, TQC], FP8, name=f"ex_{c}_{j}", tag="ex")
                    exs.append(ex)
                nc.scalar.activation(exs[-1][:, j % 2, :], st[:], AT.Exp, bias=expb[:])

        def emit_o_group(c, j2):
            otps, sps, exs = chunk_state[c]
            ex = exs[j2]
            # sums first: its 2-column LDWEIGHTS gives the weight-load
            # port slack between the 256-column v-slice loads
            nc.tensor.matmul(
                sps[:],
                ones8[:, :, 0:1],
                ex[:],
                start=(j2 == 0),
                stop=(j2 == NKP - 1),
                perf_mode=DRM,
            )
            for i in range(NHT):
                nc.tensor.matmul(
                    otps[i][:],
                    v8[j2][:, :, i * PART : (i + 1) * PART],
                    ex[:],
                    start=(j2 == 0),
                    stop=(j2 == NKP - 1),
                    perf_mode=DRM,
                )

        def emit_chunk_evicts(c, split):
            """PSUM->SBUF for otps (to fp8 ot8) and sums; split=True uses
            both DVE and ACT (kernel end), else DVE only so ACT keeps
            streaming the next chunk's exps."""
            otps, sps, _ = chunk_state[c]
            for i in range(NHT):
                dst = ot8[i // 2][:, i % 2, c * TQC : (c + 1) * TQC]
                if split and i % 2 == 1:
                    nc.scalar.copy(dst, otps[i][:])
                else:
                    nc.vector.tensor_copy(dst, otps[i][:])
            sums_sb = sums_pool.tile([1, TQC], F32, name=f"sums_{c}", tag="sums")
            if split:
                nc.vector.tensor_copy(sums_sb[:], sps[:])
            else:
                nc.scalar.activation(sums_sb[:], sps[:], AT.Identity)
            nc.sync.dma_start(sums_out[c : c + 1, :], sums_sb[:])

        for c in range(NCH):
            chunk_state[c] = (
                [
                    ot_ps.tile([PART, TQC], F32, name=f"otp_{c}_{i}", tag="otp")
                    for i in range(NHT)
                ],
                sum_ps.tile([1, TQC], F32, name=f"sump_{c}", tag="sump"),
                [],
            )
            for p in range(NKP):
                emit_scores_pair(c, p)
                if p == 1 and c > 0:
                    # previous chunk's deferred final O group + evictions
                    emit_o_group(c - 1, NKP - 1)
                    emit_chunk_evicts(c - 1, split=False)
                    del chunk_state[c - 1]
                if p > 0:
                    emit_o_group(c, p - 1)
                if c > 0 and p in (2, 3, 4, 5):
                    emit_y_tile(c - 1, p - 2)
        emit_o_group(NCH - 1, NKP - 1)
        emit_chunk_evicts(NCH - 1, split=True)

        for tl in range(TQC // PART):
            emit_y_tile(NCH - 1, tl, last=True)

    nc.compile()
    return nc


_NC_CACHE = None


def get_nc():
    global _NC_CACHE
    if _NC_CACHE is None:
        _NC_CACHE = build_nc()
    return _NC_CACHE


def make_in_maps(rgb, pose, Wq, bq, Wk, bk, Wv, bv, Wp, bp):
    rgb = np.asarray(rgb, np.float32)
    pose = np.asarray(pose, np.float32)
    Wq, bq = np.asarray(Wq, np.float32), np.asarray(bq, np.float32)
    Wk, bk = np.asarray(Wk, np.float32), np.asarray(bk, np.float32)
    Wv = np.asarray(Wv, np.float32)
    Wp = np.asarray(Wp, np.float32)

    # xq[b][c][p, d, t] = rgb[b, c*512 + t, d*128 + p] (d padded 400->512)
    xT = np.zeros((B, DRP, T), NP_FP8)
    xT[:, :DR, :] = np.swapaxes(rgb, 1, 2).astype(NP_FP8)
    xP = xT.reshape(B, NDR, PART, NCH, TQC).transpose(0, 2, 1, 3, 4)  # b,p,d,c,t
    pT = np.swapaxes(pose, 1, 2).astype(NP_FP8)
    pP = pT.reshape(B, NDP, PART, NCH, TQC).transpose(0, 2, 1, 3, 4)

    wkp = Wk.astype(NP_FP8).reshape(NDP, PART, H).transpose(1, 0, 2)
    wq8f = np.zeros((DRP, H), np.float32)
    wq8f[:DR] = Wq
    wqp = wq8f.astype(NP_FP8).reshape(NDR, PART, H).transpose(1, 0, 2)
    wvp = Wv.astype(NP_FP8).reshape(NDP, PART, H).transpose(1, 0, 2)
    wpp = Wp.astype(NP_FP8).reshape(NHT, PART, DR).transpose(1, 0, 2)
    bqbk = np.concatenate(
        [bq.reshape(NHT, PART).T, (bk * SCALE).reshape(NHT, PART).T], axis=1
    ).astype(np.float32)

    base = dict(
        wka=np.ascontiguousarray(wkp[:, :, : 2 * PART]),
        wkb=np.ascontiguousarray(wkp[:, :, 2 * PART :]),
        wva=np.ascontiguousarray(wvp[:, 0:1, :]),
        wvb=np.ascontiguousarray(wvp[:, 1:2, :]),
        wq=np.ascontiguousarray(wqp),
        wp=np.ascontiguousarray(wpp),
        bqbk=np.ascontiguousarray(bqbk),
    )
    maps = []
    for b in range(B):
        m = dict(base)
        for c in range(NCH):
            m[f"xq{c}"] = np.ascontiguousarray(xP[b, :, :, c, :])
            m[f"pq{c}"] = np.ascontiguousarray(pP[b, :, :, c, :])
        maps.append(m)
    return maps


def kernel(rgb, pose, Wq, bq, Wk, bk, Wv, bv, Wp, bp):
    rgb = np.asarray(rgb, np.float32)
    Wp_f = np.asarray(Wp, np.float32)
    bp_eff = np.asarray(bp, np.float32) + np.asarray(bv, np.float32) @ Wp_f
    in_maps = make_in_maps(rgb, pose, Wq, bq, Wk, bk, Wv, bv, Wp, bp)
    res = bass_utils.run_bass_kernel_spmd(get_nc(), in_maps, core_ids=list(range(B)))
    out = np.empty((B, T, DR), np.float32)
    for b in range(B):
        yun = np.asarray(res.results[b]["yun"]).astype(np.float32)
        sums = res.results[b]["sums_out"].reshape(T)
        out[b] = rgb[b] + bp_eff + yun / sums[:, None]
    return out


# revision 12
# speedup vs baseline: 1.0251x; 1.0050x over previous
"""Trainium2 Bass kernel for nn_CrossModalFusion (single-head cross attention).

Per-batch-element cross attention, data-parallel over B=8 across 8 NeuronCores.

Per core (T=2048, D_RGB=400, D_POSE=256, H=512):
    q = rgb @ Wq + bq ; k = pose @ Wk + bk ; v = pose @ Wv
    S = q @ k.T / sqrt(H) ; A = exp(S) (no max-sub needed; scores are O(1))
    y = rgb + bp + bv@Wp + (A @ v) @ Wp / rowsum(A)

Layout strategy (zero on-device transposes):
  - host pre-permutes every tensor into its exact SBUF layout so each DMA is
    a flat contiguous copy; inputs ordered across the sync/scalar/gpsimd
    queues to match first-use times (per-queue DMA sustains only ~50GB/s)
  - warm-up matmuls on a memset tile burn the PE p-state ramp (0.65->2.4GHz)
    inside the input-DMA window
  - qT,kT computed h-major [h,t]; scores computed transposed ST=[tk,tq]
  - exp(ST - ln 32) on ACT (the 1/32 keeps unnormalized O inside fp8e4m3
    range; it cancels in the normalization); O^T accumulated via lhsT=v
    (natural layout), rhs=exp(ST); row-sums via a ones-vector matmul
  - every matmul uses fp8e4m3 operands with perf_mode=DoubleRow (pairs of
    128-row k-subtiles), fp32 PSUM accumulation everywhere
  - PSUM->SBUF evictions (fp32->fp8 casts on DVE/ACT, ~0.7us each) are the
    projection-phase bottleneck, so only kT, v tiles 0-7 and qT chunk 0 are
    projected up front; v tiles 8-15 and qT chunks 1-3 are interleaved into
    attention chunks where DVE has slack and the PE is the limiter
  - the O/sums accumulation for score-pair p is emitted after the scores of
    pair p+1 (one-pair software pipeline); the final pair's O group lands
    after the NEXT chunk's first two score tiles, so the PE never head-of-line
    blocks on the chunk's last exp
  - device returns unnormalized (A@v)@Wp in BF16 (halves the exposed tail
    DMA) and rowsum(A) in fp32; the host applies
    y = rgb + bp' + yun/sums in fp32 (0.1% of the FLOPs, exact division)
"""

import sys

if "/opt/trn_rl_repo" not in sys.path:
    sys.path.insert(0, "/opt/trn_rl_repo")

from contextlib import ExitStack

import ml_dtypes
import numpy as np

import concourse.mybir as mybir
import concourse.tile as tile
from concourse import bacc, bass_utils

FP8 = mybir.dt.float8e4
F32 = mybir.dt.float32
BF16 = mybir.dt.bfloat16
NP_FP8 = ml_dtypes.float8_e4m3

B, T, DR, DP, H = 8, 2048, 400, 256, 512
PART = 128
DRP = 512                # rgb feature dim padded to 4*128
TQC = 512                # tq chunk width (max PSUM free dim)
NCH = T // TQC           # 4 chunks
NTK = T // PART          # 16 key tiles
NKP = NTK // 2           # 8 key tile pairs (DoubleRow)
NHT = H // PART          # 4 h tiles
NHP = NHT // 2           # 2 h tile pairs
NDR = DRP // PART        # 4 padded-rgb d tiles
NDRP = NDR // 2          # 2 pairs
NDP = DP // PART         # 2 pose d tiles
SCALE = float(1.0 / np.sqrt(np.float32(H)))
EXP_BIAS = float(-np.log(32.0))

AT = mybir.ActivationFunctionType
OP = mybir.AluOpType
DRM = mybir.MatmulPerfMode.DoubleRow


def build_nc():
    nc = bacc.Bacc(
        "TRN2",
        target_bir_lowering=False,
        debug=False,
        enable_asserts=False,
        num_devices=8,
    )
    # all inputs arrive pre-permuted from the host: flat contiguous DMAs only
    xq_d = [
        nc.dram_tensor(f"xq{c}", (PART, NDR, TQC), FP8, kind="ExternalInput").ap()
        for c in range(NCH)
    ]
    pq_d = [
        nc.dram_tensor(f"pq{c}", (PART, NDP, TQC), FP8, kind="ExternalInput").ap()
        for c in range(NCH)
    ]
    wka_d = nc.dram_tensor("wka", (PART, NDP, 2 * PART), FP8, kind="ExternalInput").ap()
    wkb_d = nc.dram_tensor("wkb", (PART, NDP, 2 * PART), FP8, kind="ExternalInput").ap()
    wva_d = nc.dram_tensor("wva", (PART, 1, H), FP8, kind="ExternalInput").ap()
    wvb_d = nc.dram_tensor("wvb", (PART, 1, H), FP8, kind="ExternalInput").ap()
    wq_d = nc.dram_tensor("wq", (PART, NDR, H), FP8, kind="ExternalInput").ap()
    wp_d = nc.dram_tensor("wp", (PART, NHT, DR), FP8, kind="ExternalInput").ap()
    bqbk_d = nc.dram_tensor("bqbk", (PART, 2 * NHT), F32, kind="ExternalInput").ap()
    yun = nc.dram_tensor("yun", (T, DR), BF16, kind="ExternalOutput").ap()
    sums_out = nc.dram_tensor("sums_out", (NCH, TQC), F32, kind="ExternalOutput").ap()

    with tile.TileContext(nc) as tc, ExitStack() as ctx:
        const = ctx.enter_context(tc.tile_pool(name="const", bufs=1))
        mm_ps = ctx.enter_context(tc.tile_pool(name="mm_ps", bufs=3, space="PSUM"))
        ot_ps = ctx.enter_context(tc.tile_pool(name="ot_ps", bufs=4, space="PSUM"))
        sum_ps = ctx.enter_context(tc.tile_pool(name="sum_ps", bufs=1, space="PSUM"))
        ex_pool = ctx.enter_context(tc.tile_pool(name="ex_pool", bufs=10))
        sums_pool = ctx.enter_context(tc.tile_pool(name="sums_pool", bufs=2))
        ysb_pool = ctx.enter_context(tc.tile_pool(name="ysb_pool", bufs=4))

        # ---- persistent inputs ----
        # flat contiguous DMAs, ordered per queue by first-use time:
        #   scalar: wka, wv half a, wq, xq2, wp
        #   sync:   pq0..pq3, xq3
        #   gpsimd: bqbk, wkb, wv half b, xq0, xq1
        wk8 = [const.tile([PART, NDP, 2 * PART], FP8, name=f"wk8_{i}") for i in range(2)]
        wv8 = const.tile([PART, NDP, H], FP8, name="wv8")
        wq8 = const.tile([PART, NDR, H], FP8, name="wq8")
        wp8 = const.tile([PART, NHT, DR], FP8, name="wp8")
        p8 = [const.tile([PART, NDP, TQC], FP8, name=f"p8_{c}") for c in range(NCH)]
        x8 = [const.tile([PART, NDR, TQC], FP8, name=f"x8_{c}") for c in range(NCH)]
        bqbk = const.tile([PART, 2 * NHT], F32, name="bqbk")

        nc.scalar.dma_start(wk8[0][:], wka_d[:])
        nc.scalar.dma_start(wv8[:, 0:1, :], wva_d[:])
        nc.scalar.dma_start(wq8[:], wq_d[:])
        nc.scalar.dma_start(x8[2][:], xq_d[2][:])
        nc.scalar.dma_start(wp8[:], wp_d[:])

        for c in range(NCH):
            nc.sync.dma_start(p8[c][:], pq_d[c][:])
        nc.sync.dma_start(x8[3][:], xq_d[3][:])

        nc.gpsimd.dma_start(bqbk[:], bqbk_d[:])
        nc.gpsimd.dma_start(wk8[1][:], wkb_d[:])
        nc.gpsimd.dma_start(wv8[:, 1:2, :], wvb_d[:])
        nc.gpsimd.dma_start(x8[0][:], xq_d[0][:])
        nc.gpsimd.dma_start(x8[1][:], xq_d[1][:])

        ones8 = const.tile([PART, 2, PART], FP8, name="ones8")
        nc.vector.memset(ones8[:], 1.0)
        expb = const.tile([PART, 1], F32, name="expb")
        nc.vector.memset(expb[:], EXP_BIAS)

        # ---- persistent intermediates (fp8 DoubleRow pair layouts) ----
        # qT8[i2][p, s, t] = q[h = i2*256 + s*128 + p, t]
        qT8 = [const.tile([PART, 2, T], FP8, name=f"qT8_{i}") for i in range(NHP)]
        kT8 = [const.tile([PART, 2, T], FP8, name=f"kT8_{i}") for i in range(NHP)]
        # v8[j2][p, s, h] = v[t = j2*256 + s*128 + p, h]
        v8 = [const.tile([PART, 2, H], FP8, name=f"v8_{j}") for j in range(NKP)]
        # ot8[i2][p, s, t] = O[h = i2*256 + s*128 + p, t] (unnormalized, /32)
        ot8 = [const.tile([PART, 2, T], FP8, name=f"ot8_{i}") for i in range(NHP)]

        # ---- PE warm-up: burn the p-state ramp inside the DMA window ----
        wps = mm_ps.tile([PART, PART], F32, name="warm", tag="mmps")
        for w in range(14):
            nc.tensor.matmul(
                wps[:], ones8[:], ones8[:], start=(w == 0), stop=(w == 13),
                perf_mode=DRM,
            )
        warm_sink = const.tile([PART, PART], BF16, name="warm_sink")
        nc.vector.tensor_copy(warm_sink[:], wps[:])

        def evict_biased(n, dst, ps, bias_ap, scale):
            """PSUM->SBUF cast with scale*x+bias, alternating DVE/ACT."""
            if n % 2 == 0:
                if scale == 1.0:
                    nc.vector.tensor_scalar_add(dst, ps, bias_ap)
                else:
                    nc.vector.tensor_scalar(
                        dst, ps, scale, bias_ap, op0=OP.mult, op1=OP.add
                    )
            else:
                nc.scalar.activation(dst, ps, AT.Identity, bias=bias_ap, scale=scale)

        def emit_v_tile(j, dve_evict):
            """v[t,h] = pT[d,t].T @ Wv[d,h] -> fp8 for key tile j."""
            ps = mm_ps.tile([PART, H], F32, name=f"vps_{j}", tag="mmps")
            nc.tensor.matmul(
                ps[:],
                p8[j // 4][:, :, (j % 4) * PART : (j % 4 + 1) * PART],
                wv8[:],
                start=True,
                stop=True,
                perf_mode=DRM,
            )
            if dve_evict:
                nc.vector.tensor_copy(v8[j // 2][:, j % 2, :], ps[:])
            else:
                nc.scalar.copy(v8[j // 2][:, j % 2, :], ps[:])

        def emit_qT_half(c, half, dve_evict=False):
            """qT[h,t] = Wq[d,h].T @ xT[d,t] + bq -> fp8, h tiles 2*half..2*half+1."""
            for i in (2 * half, 2 * half + 1):
                ps = mm_ps.tile([PART, TQC], F32, name=f"qps_{i}_{c}", tag="mmps")
                for d2 in range(NDRP):
                    nc.tensor.matmul(
                        ps[:],
                        wq8[:, 2 * d2 : 2 * d2 + 2, i * PART : (i + 1) * PART],
                        x8[c][:, 2 * d2 : 2 * d2 + 2, :],
                        start=(d2 == 0),
                        stop=(d2 == NDRP - 1),
                        perf_mode=DRM,
                    )
                dst = qT8[i // 2][:, i % 2, c * TQC : (c + 1) * TQC]
                if dve_evict:
                    nc.vector.tensor_scalar_add(dst, ps[:], bqbk[:, i : i + 1])
                else:
                    evict_biased(c * NHT + i + 1, dst, ps[:], bqbk[:, i : i + 1], 1.0)

        # ---- phase B: front-loaded projections ----
        # kT (all, scores need every key tile), v tiles 0-7, qT chunk 0.
        # kT chunk c and v tiles interleave to track the pose-quarter DMA
        # arrivals; the rest of v and qT chunks 1-3 are deferred into the
        # attention chunks (phase B is eviction-bound, attention is PE-bound).
        for c in range(NCH):
            for i in range(NHT):
                ps = mm_ps.tile([PART, TQC], F32, name=f"kps_{i}_{c}", tag="mmps")
                nc.tensor.matmul(
                    ps[:],
                    wk8[i // 2][:, :, (i % 2) * PART : (i % 2 + 1) * PART],
                    p8[c][:],
                    start=True,
                    stop=True,
                    perf_mode=DRM,
                )
                evict_biased(
                    c * NHT + i,
                    kT8[i // 2][:, i % 2, c * TQC : (c + 1) * TQC],
                    ps[:],
                    bqbk[:, NHT + i : NHT + i + 1],
                    SCALE,
                )
            for jl in range(2):
                j = c * 2 + jl
                emit_v_tile(j, dve_evict=(j % 2 == 0))
        emit_qT_half(0, 0)
        emit_qT_half(0, 1)

        # ---- phase C: attention, chunked over tq ----
        # Software pipeline: the O/sums group for score-pair p is emitted
        # after pair p+1's scores; the last pair's group lands after the next
        # chunk's first two score tiles.  Deferred projections and the
        # previous chunk's output tiles ride in the per-pair slots.
        def emit_y_tile(c, tl, last=False):
            tg = c * (TQC // PART) + tl
            yp = mm_ps.tile([PART, DR], F32, name=f"yp_{tg}", tag="mmps")
            for i2 in range(NHP):
                nc.tensor.matmul(
                    yp[:],
                    ot8[i2][:, :, tg * PART : (tg + 1) * PART],
                    wp8[:, 2 * i2 : 2 * i2 + 2, :],
                    start=(i2 == 0),
                    stop=(i2 == NHP - 1),
                    perf_mode=DRM,
                )
            ysb = ysb_pool.tile([PART, DR], BF16, name=f"ysb_{tg}", tag="ysb")
            dst = yun[tg * PART : (tg + 1) * PART, :]
            if last:
                # final chunk: ACT only evicts t13 (keeps its queue clear of
                # descriptors); halve each transfer across two queues
                if tl == 1:
                    nc.scalar.copy(ysb[:], yp[:])
                else:
                    nc.vector.tensor_copy(ysb[:], yp[:])
                e0, e1 = ((nc.sync, nc.gpsimd), (nc.scalar, nc.sync),
                          (nc.gpsimd, nc.sync), (nc.sync, nc.gpsimd))[tl]
                e0.dma_start(dst[0 : PART // 2, :], ysb[0 : PART // 2, :])
                e1.dma_start(dst[PART // 2 : PART, :], ysb[PART // 2 : PART, :])
            else:
                # mid-kernel: DVE evicts (ACT is exp-saturated); DMAs on the
                # sync/gpsimd queues so no descriptor lands on ACT's stream
                nc.vector.tensor_copy(ysb[:], yp[:])
                (nc.sync, nc.gpsimd)[tl % 2].dma_start(dst, ysb[:])

        chunk_state = {}  # c -> (otps, sps, exs)

        def emit_scores_pair(c, p):
            exs = chunk_state[c][2]
            for j in (2 * p, 2 * p + 1):
                st = mm_ps.tile([PART, TQC], F32, name=f"st_{c}_{j}", tag="mmps")
                for i2 in range(NHP):
                    nc.tensor.matmul(
                        st[:],
                        kT8[i2][:, :, j * PART : (j + 1) * PART],
                        qT8[i2][:, :, c * TQC : (c + 1) * TQC],
                        start=(i2 == 0),
                        stop=(i2 == NHP - 1),
                        perf_mode=DRM,
                    )
                if j % 2 == 0:
                    ex = ex_pool.tile([PART, 2, TQC], FP8, name=f"ex_{c}_{j}", tag="ex")
                    exs.append(ex)
                nc.scalar.activation(exs[-1][:, j % 2, :], st[:], AT.Exp, bias=expb[:])

        def emit_o_group(c, j2):
            otps, sps, exs = chunk_state[c]
            ex = exs[j2]
            # sums first: its 2-column LDWEIGHTS gives the weight-load
            # port slack between the 256-column v-slice loads
            nc.tensor.matmul(
                sps[:],
                ones8[:, :, 0:1],
                ex[:],
                start=(j2 == 0),
                stop=(j2 == NKP - 1),
                perf_mode=DRM,
            )
            for i in range(NHT):
                nc.tensor.matmul(
                    otps[i][:],
                    v8[j2][:, :, i * PART : (i + 1) * PART],
                    ex[:],
                    start=(j2 == 0),
                    stop=(j2 == NKP - 1),
                    perf_mode=DRM,
                )

        def emit_chunk_evicts(c, split):
            """PSUM->SBUF for otps (to fp8 ot8) and sums; split=True also
            uses ACT for half the casts (kernel end), else DVE so ACT keeps
            streaming the next chunk's exps (sums is one short ACT op)."""
            otps, sps, _ = chunk_state[c]
            for i in range(NHT):
                dst = ot8[i // 2][:, i % 2, c * TQC : (c + 1) * TQC]
                if split and i % 2 == 0:
                    nc.scalar.copy(dst, otps[i][:])
                else:
                    nc.vector.tensor_copy(dst, otps[i][:])
            sums_sb = sums_pool.tile([1, TQC], F32, name=f"sums_{c}", tag="sums")
            nc.scalar.copy(sums_sb[:], sps[:])
            nc.sync.dma_start(sums_out[c : c + 1, :], sums_sb[:])

        for c in range(NCH):
            chunk_state[c] = (
                [
                    ot_ps.tile([PART, TQC], F32, name=f"otp_{c}_{i}", tag="otp")
                    for i in range(NHT)
                ],
                sum_ps.tile([1, TQC], F32, name=f"sump_{c}", tag="sump"),
                [],
            )
            for p in range(NKP):
                emit_scores_pair(c, p)
                if p == 1 and c > 0:
                    # previous chunk's deferred final O group + evictions
                    emit_o_group(c - 1, NKP - 1)
                    emit_chunk_evicts(c - 1, split=False)
                    del chunk_state[c - 1]
                if p > 0:
                    emit_o_group(c, p - 1)
                # deferred projections / previous chunk's output tiles
                if c == 0:
                    if p in (1, 2, 3, 4):
                        emit_v_tile(2 * p + 6, dve_evict=True)
                        emit_v_tile(2 * p + 7, dve_evict=True)
                    elif p in (5, 6):
                        emit_qT_half(1, p - 5, dve_evict=True)
                else:
                    if c < NCH - 1 and p in (2, 3):
                        emit_qT_half(c + 1, p - 2, dve_evict=True)
                    if p in (2, 3, 4, 5):
                        emit_y_tile(c - 1, p - 2)
        emit_o_group(NCH - 1, NKP - 1)
        emit_chunk_evicts(NCH - 1, split=True)

        for tl in range(TQC // PART):
            emit_y_tile(NCH - 1, tl, last=True)

    nc.compile()
    return nc


_NC_CACHE = None


def get_nc():
    global _NC_CACHE
    if _NC_CACHE is None:
        _NC_CACHE = build_nc()
    return _NC_CACHE


def make_in_maps(rgb, pose, Wq, bq, Wk, bk, Wv, bv, Wp, bp):
    rgb = np.asarray(rgb, np.float32)
    pose = np.asarray(pose, np.float32)
    Wq, bq = np.asarray(Wq, np.float32), np.asarray(bq, np.float32)
    Wk, bk = np.asarray(Wk, np.float32), np.asarray(bk, np.float32)
    Wv = np.asarray(Wv, np.float32)
    Wp = np.asarray(Wp, np.float32)

    # xq[b][c][p, d, t] = rgb[b, c*512 + t, d*128 + p] (d padded 400->512)
    xT = np.zeros((B, DRP, T), NP_FP8)
    xT[:, :DR, :] = np.swapaxes(rgb, 1, 2).astype(NP_FP8)
    xP = xT.reshape(B, NDR, PART, NCH, TQC).transpose(0, 2, 1, 3, 4)  # b,p,d,c,t
    pT = np.swapaxes(pose, 1, 2).astype(NP_FP8)
    pP = pT.reshape(B, NDP, PART, NCH, TQC).transpose(0, 2, 1, 3, 4)

    wkp = Wk.astype(NP_FP8).reshape(NDP, PART, H).transpose(1, 0, 2)
    wq8f = np.zeros((DRP, H), np.float32)
    wq8f[:DR] = Wq
    wqp = wq8f.astype(NP_FP8).reshape(NDR, PART, H).transpose(1, 0, 2)
    wvp = Wv.astype(NP_FP8).reshape(NDP, PART, H).transpose(1, 0, 2)
    wpp = Wp.astype(NP_FP8).reshape(NHT, PART, DR).transpose(1, 0, 2)
    bqbk = np.concatenate(
        [bq.reshape(NHT, PART).T, (bk * SCALE).reshape(NHT, PART).T], axis=1
    ).astype(np.float32)

    base = dict(
        wka=np.ascontiguousarray(wkp[:, :, : 2 * PART]),
        wkb=np.ascontiguousarray(wkp[:, :, 2 * PART :]),
        wva=np.ascontiguousarray(wvp[:, 0:1, :]),
        wvb=np.ascontiguousarray(wvp[:, 1:2, :]),
        wq=np.ascontiguousarray(wqp),
        wp=np.ascontiguousarray(wpp),
        bqbk=np.ascontiguousarray(bqbk),
    )
    maps = []
    for b in range(B):
        m = dict(base)
        for c in range(NCH):
            m[f"xq{c}"] = np.ascontiguousarray(xP[b, :, :, c, :])
            m[f"pq{c}"] = np.ascontiguousarray(pP[b, :, :, c, :])
        maps.append(m)
    return maps


def kernel(rgb, pose, Wq, bq, Wk, bk, Wv, bv, Wp, bp):
    rgb = np.asarray(rgb, np.float32)
    Wp_f = np.asarray(Wp, np.float32)
    bp_eff = np.asarray(bp, np.float32) + np.asarray(bv, np.float32) @ Wp_f
    in_maps = make_in_maps(rgb, pose, Wq, bq, Wk, bk, Wv, bv, Wp, bp)
    res = bass_utils.run_bass_kernel_spmd(get_nc(), in_maps, core_ids=list(range(B)))
    out = np.empty((B, T, DR), np.float32)
    for b in range(B):
        yun = np.asarray(res.results[b]["yun"]).astype(np.float32)
        sums = res.results[b]["sums_out"].reshape(T)
        out[b] = rgb[b] + bp_eff + yun / sums[:, None]
    return out
